# revision 6
# baseline (speedup 1.0000x reference)
"""Trainium2 Bass kernel for nn_BackbonePointNet (3-layer PointNet-style GNN).

Sharding: destination nodes across 8 cores (12.5K nodes / 200K edges each).
Per layer l (factored edge MLP):
    pre(e) = u_l[src_e] + v_l[dst_e]        (v holds the negated dst part)
    msg(e) = relu(pre) @ wb_l               (bias bb_l folded in after max)
    h(i)   = relu(max_{e->i} msg(e) + bb_l)
with u_l = concat(h_{l-1}, 1) @ [wa_h; ba] + pos @ wa_p  computed per-core
for local nodes then AllGather-replicated in bf16.  Edge phase per
1024-edge macro-tile: 8 indirect-DMA row gathers of u[src] (128 rows each),
transposed into PSUM via identity matmuls (accumulating on top of the
v-selector matmul), ACT relu -> bf16, second linear on PE, segment max via
strided tensor_reduce written straight into the transposed h accumulator,
which directly feeds the next layer's u matmuls.  Pooling (segment mean,
sorted batch) and the 2-layer regressor + sigmoid are O(B*C) and run on
host in f64/f32.
"""

import numpy as np
import ml_dtypes

N = 100_000
E = 16 * N
B = 64
NCORES = 8

_BF = ml_dtypes.bfloat16
_CACHE = {}


# --------------------------------------------------------------------------
# device program
# --------------------------------------------------------------------------

def _build_nc(n_nodes, n_loc_pad, d_grp, et, n_cores):
    from concourse import bass, mybir, tile  # noqa: F401
    import concourse.bacc as bacc

    BF16 = mybir.dt.bfloat16
    F32 = mybir.dt.float32
    AF = mybir.ActivationFunctionType

    e_loc = n_loc_pad * d_grp
    n_tiles = e_loc // et
    npt = et // d_grp                      # nodes per macro tile
    chunks = et // 128
    n_loc = n_nodes // n_cores

    nc = bacc.Bacc("TRN2", target_bir_lowering=False, debug=False,
                   num_devices=n_cores)

    # ---- external inputs ----
    u1_full = nc.dram_tensor("u1_full", [n_nodes, 64], BF16, kind="ExternalInput")
    gidx = nc.dram_tensor("gidx", [128, n_tiles * chunks], mybir.dt.int32,
                          kind="ExternalInput")
    # packed bf16 constants: [id128 | sel | w2h(65) | w2p(3) | w3h(65) |
    #                         w3p(3) | wb1 | wb2 | wb3]
    CW = 128 + et + 64 + 64 + 128 + 128 + 64 + 64 + 128
    cblob = nc.dram_tensor("cblob", [128, CW], BF16, kind="ExternalInput")
    fblob = nc.dram_tensor("fblob", [128, 3], F32, kind="ExternalInput")
    posT = nc.dram_tensor("posT", [3, n_loc_pad], BF16, kind="ExternalInput")
    v1 = nc.dram_tensor("v1", [n_loc_pad, 64], BF16, kind="ExternalInput")
    v2 = nc.dram_tensor("v2", [n_loc_pad, 64], BF16, kind="ExternalInput")
    v3 = nc.dram_tensor("v3", [n_loc_pad, 128], BF16, kind="ExternalInput")
    hT3_out = nc.dram_tensor("hT3", [128, n_loc_pad], F32, kind="ExternalOutput")

    # internal dram for u slices / replicated tables
    u2_slice = nc.dram_tensor("u2_slice", [n_loc, 64], BF16, kind="Internal")
    u3_slice = nc.dram_tensor("u3_slice", [n_loc, 128], BF16, kind="Internal")
    u2_full = nc.dram_tensor("u2_full", [n_nodes, 64], BF16, kind="Internal",
                             addr_space="Shared")
    u3_full = nc.dram_tensor("u3_full", [n_nodes, 128], BF16, kind="Internal",
                             addr_space="Shared")

    with tile.TileContext(nc) as tc:
        with tc.tile_pool(name="const", bufs=1) as cp, \
             tc.tile_pool(name="gath", bufs=16) as gp, \
             tc.tile_pool(name="work", bufs=3) as wp, \
             tc.tile_pool(name="out", bufs=2) as op, \
             tc.tile_pool(name="hbuf", bufs=1) as hp, \
             tc.tile_pool(name="psum", bufs=2, space="PSUM") as pp:

            # ---- resident constants ----
            gidx_t = cp.tile([128, n_tiles * chunks], mybir.dt.int32)
            nc.sync.dma_start(out=gidx_t[:], in_=gidx[:])
            cb = cp.tile([128, CW], BF16)
            nc.sync.dma_start(out=cb[:], in_=cblob[:])
            fb = cp.tile([128, 3], F32)
            nc.sync.dma_start(out=fb[:], in_=fblob[:])
            posT_t = cp.tile([3, n_loc_pad], BF16)
            nc.sync.dma_start(out=posT_t[:], in_=posT[:])

            o_id = 0
            o_sel = o_id + 128
            o_w2h = o_sel + et
            o_w2p = o_w2h + 64
            o_w3h = o_w2p + 64
            o_w3p = o_w3h + 128
            o_wb1 = o_w3p + 128
            o_wb2 = o_wb1 + 64
            o_wb3 = o_wb2 + 64
            id_ap = cb[:, o_id:o_id + 128]
            w_ap = {
                "w2h": cb[0:65, o_w2h:o_w2h + 64],
                "w2p": cb[0:3, o_w2p:o_w2p + 64],
                "w3h": cb[0:65, o_w3h:o_w3h + 128],
                "w3p": cb[0:3, o_w3p:o_w3p + 128],
                "wb1": cb[0:64, o_wb1:o_wb1 + 64],
                "wb2": cb[0:64, o_wb2:o_wb2 + 64],
                "wb3": cb[0:128, o_wb3:o_wb3 + 128],
            }
            bb_ap = {"bb1": fb[0:64, 0:1], "bb2": fb[0:64, 1:2],
                     "bb3": fb[0:128, 2:3]}

            v_t = {}
            for name, h, c in (("v1", v1, 64), ("v2", v2, 64), ("v3", v3, 128)):
                t = cp.tile([128, (n_loc_pad // 128) * c], BF16, tag=name)
                nc.sync.dma_start(
                    out=t[:].rearrange("p (m c) -> p m c", c=c),
                    in_=h[:].rearrange("(m p) c -> p m c", p=128))
                v_t[name] = (t, c)

            hT1 = hp.tile([65, n_loc_pad], BF16, tag="hT1")
            hT2 = hp.tile([65, n_loc_pad], BF16, tag="hT2")
            hTr = hp.tile([128, n_loc_pad], BF16, tag="hTraw")
            nc.vector.memset(hT1[64:65, :], 1.0)
            nc.vector.memset(hT2[64:65, :], 1.0)

            def edge_phase(u_src_ap, v_name, wb_name, c_in, c_out):
                vt, vc = v_t[v_name]
                vv = vt[:].rearrange("p (m c) -> p m c", c=vc)
                for t in range(n_tiles):
                    gts = []
                    for c in range(chunks):
                        gt = gp.tile([128, c_in], BF16, tag="g")
                        nc.gpsimd.indirect_dma_start(
                            out=gt[:], out_offset=None,
                            in_=u_src_ap,
                            in_offset=bass.IndirectOffsetOnAxis(
                                ap=gidx_t[:, t * chunks + c: t * chunks + c + 1],
                                axis=0),
                        )
                        gts.append(gt)
                    pre = pp.tile([c_in, et], F32, tag="pre", space="PSUM")
                    r0 = (t * npt) % 128
                    m0 = (t * npt) // 128
                    vslice = vv[r0:r0 + npt, m0:m0 + 1, :]
                    cpH = chunks // 2
                    for h in range(2):
                        sel_ap = cb[r0:r0 + npt, o_sel + h * 512:o_sel + h * 512 + 512]
                        nc.tensor.matmul(out=pre[:, h * 512:h * 512 + 512],
                                         lhsT=vslice, rhs=sel_ap,
                                         start=True, stop=False)
                        for j in range(cpH):
                            c = h * cpH + j
                            nc.tensor.matmul(
                                out=pre[:, c * 128:(c + 1) * 128],
                                lhsT=gts[c][:], rhs=id_ap,
                                start=False, stop=(j == cpH - 1),
                                skip_group_check=True)
                    prs = wp.tile([128, et], BF16, tag="prs")
                    nc.scalar.activation(out=prs[:c_in, :], in_=pre[:],
                                         func=AF.Relu)
                    msg = pp.tile([c_out, et], F32, tag="msg", space="PSUM")
                    for h in range(2):
                        nc.tensor.matmul(out=msg[:, h * 512:h * 512 + 512],
                                         lhsT=w_ap[wb_name],
                                         rhs=prs[:c_in, h * 512:h * 512 + 512],
                                         start=True, stop=True)
                    nc.vector.tensor_reduce(
                        out=hTr[:c_out, t * npt:(t + 1) * npt],
                        in_=msg[:].rearrange("p (n k) -> p n k", k=d_grp),
                        axis=mybir.AxisListType.X, op=mybir.AluOpType.max)

            def bias_relu(c_out, bb_name, dst):
                nc.scalar.activation(out=dst[:c_out, :], in_=hTr[:c_out, :],
                                     func=AF.Relu, bias=bb_ap[bb_name],
                                     scale=1.0)

            def u_phase(hT, wh_name, wp_name, c_out, u_slice):
                for m in range(n_loc_pad // 128):
                    ps = pp.tile([128, et], F32, tag="pre", space="PSUM")
                    nc.tensor.matmul(
                        out=ps[:, :c_out],
                        lhsT=hT[:, m * 128:(m + 1) * 128],
                        rhs=w_ap[wh_name], start=True, stop=False)
                    nc.tensor.matmul(
                        out=ps[:, :c_out],
                        lhsT=posT_t[:, m * 128:(m + 1) * 128],
                        rhs=w_ap[wp_name], start=False, stop=True,
                        skip_group_check=True)
                    us = wp.tile([128, 128], BF16, tag="us")
                    nc.scalar.activation(out=us[:, :c_out], in_=ps[:, :c_out],
                                         func=AF.Copy)
                    lo = m * 128
                    hi = min((m + 1) * 128, n_loc)
                    if hi > lo:
                        nc.sync.dma_start(out=u_slice[lo:hi, :],
                                          in_=us[:hi - lo, :c_out])

            # ---------------- layer 1 ----------------
            edge_phase(u1_full[:], "v1", "wb1", 64, 64)
            bias_relu(64, "bb1", hT1)

            # ---------------- layer 2 ----------------
            u_phase(hT1, "w2h", "w2p", 64, u2_slice)
            nc.gpsimd.collective_compute(
                "AllGather", mybir.AluOpType.bypass,
                replica_groups=[list(range(n_cores))],
                ins=[u2_slice[:]], outs=[u2_full[:]])
            edge_phase(u2_full[:], "v2", "wb2", 64, 64)
            bias_relu(64, "bb2", hT2)

            # ---------------- layer 3 ----------------
            u_phase(hT2, "w3h", "w3p", 128, u3_slice)
            nc.gpsimd.collective_compute(
                "AllGather", mybir.AluOpType.bypass,
                replica_groups=[list(range(n_cores))],
                ins=[u3_slice[:]], outs=[u3_full[:]])
            edge_phase(u3_full[:], "v3", "wb3", 128, 128)

            # final h3 = relu(raw + bb3) -> f32 output, in strips
            ns = 8
            strip = n_loc_pad // ns
            for s in range(ns):
                h3t = op.tile([128, strip], F32, tag="h3")
                nc.scalar.activation(out=h3t[:], in_=hTr[:, s * strip:(s + 1) * strip],
                                     func=AF.Relu, bias=bb_ap["bb3"], scale=1.0)
                nc.sync.dma_start(out=hT3_out[:, s * strip:(s + 1) * strip],
                                  in_=h3t[:])

    nc.compile()
    return nc


# --------------------------------------------------------------------------
# host side
# --------------------------------------------------------------------------

def _next_pow2_ge(x, lo=16):
    d = lo
    while d < x:
        d *= 2
    return d


def _prep(pos, edge_index, weights, n_cores):
    n_nodes = pos.shape[0]
    src = edge_index[0].astype(np.int64)
    dst = edge_index[1].astype(np.int64)
    e_tot = src.shape[0]

    canonical = (e_tot == 16 * n_nodes) and np.array_equal(
        dst, np.repeat(np.arange(n_nodes, dtype=np.int64), e_tot // n_nodes))

    if canonical and e_tot // n_nodes == 16:
        d_grp = 16
        slot_src = src.reshape(n_nodes, 16)
        deg0 = None
    else:
        order = np.argsort(dst, kind="stable")
        s_sorted = src[order]
        counts = np.bincount(dst, minlength=n_nodes)
        d_grp = _next_pow2_ge(int(counts.max()) if e_tot else 16)
        starts = np.concatenate([[0], np.cumsum(counts)])
        slot_src = np.zeros((n_nodes, d_grp), np.int64)
        idx = np.arange(d_grp)
        for i in range(n_nodes):
            c = counts[i]
            if c:
                row = s_sorted[starts[i]:starts[i] + c]
                slot_src[i] = row[idx % c]
        deg0 = counts == 0

    n_loc = n_nodes // n_cores
    et = 1024
    npt = et // d_grp
    n_loc_pad = int(np.ceil(n_loc / 128) * 128)

    sel_np = np.zeros((npt, et), np.float32)
    for k in range(npt):
        sel_np[k, k * d_grp:(k + 1) * d_grp] = 1.0

    w = weights
    u1_full = (pos @ (w['w1a'][:3] + w['w1a'][3:6]) + w['b1a']).astype(_BF)

    CW = 128 + et + 64 + 64 + 128 + 128 + 64 + 64 + 128
    cblob = np.zeros((128, CW), np.float32)
    o = 0
    cblob[:128, o:o + 128] = np.eye(128); o += 128
    cblob[:, o:o + et] = np.tile(sel_np, (128 // npt, 1)); o += et
    cblob[:65, o:o + 64] = np.concatenate([w['w2a'][:64], w['b2a'][None]], 0); o += 64
    cblob[:3, o:o + 64] = w['w2a'][64:67]; o += 64
    cblob[:65, o:o + 128] = np.concatenate([w['w3a'][:64], w['b3a'][None]], 0); o += 128
    cblob[:3, o:o + 128] = w['w3a'][64:67]; o += 128
    cblob[:64, o:o + 64] = w['w1b']; o += 64
    cblob[:64, o:o + 64] = w['w2b']; o += 64
    cblob[:128, o:o + 128] = w['w3b']; o += 128
    fblob = np.zeros((128, 3), np.float32)
    fblob[:64, 0] = w['b1b']
    fblob[:64, 1] = w['b2b']
    fblob[:128, 2] = w['b3b']

    common = dict(u1_full=u1_full, cblob=cblob.astype(_BF), fblob=fblob)

    chunks = et // 128
    n_tiles = n_loc_pad * d_grp // et
    per_core = []
    for c in range(n_cores):
        lo = c * n_loc
        pos_l = np.zeros((n_loc_pad, 3), np.float32)
        pos_l[:n_loc] = pos[lo:lo + n_loc]
        posT = pos_l.T.astype(_BF)
        vs = {
            "v1": (-(pos_l @ w['w1a'][3:6])).astype(_BF),
            "v2": (-(pos_l @ w['w2a'][64:67])).astype(_BF),
            "v3": (-(pos_l @ w['w3a'][64:67])).astype(_BF),
        }
        ss = np.zeros((n_loc_pad, d_grp), np.int64)
        ss[:n_loc] = slot_src[lo:lo + n_loc]
        gidx = ss.reshape(-1).reshape(n_tiles, chunks, 128).transpose(2, 0, 1)
        gidx = np.ascontiguousarray(gidx.reshape(128, n_tiles * chunks),
                                    dtype=np.int32)
        per_core.append(dict(posT=posT, gidx=gidx, **vs))

    cfg = dict(n_nodes=n_nodes, n_loc_pad=n_loc_pad, d_grp=d_grp, et=et,
               n_cores=n_cores)
    meta = dict(n_loc=n_loc, deg0=deg0)
    return cfg, common, per_core, meta


def kernel(pos, edge_index, batch, timestep,
           w1a, b1a, w1b, b1b, w2a, b2a, w2b, b2b,
           w3a, b3a, w3b, b3b, wr1, br1, wr2, br2):
    from concourse import bass_utils

    pos = np.asarray(pos, np.float32)
    edge_index = np.asarray(edge_index, np.int32)
    batch = np.asarray(batch, np.int32)
    W = {k: np.asarray(v, np.float32) for k, v in dict(
        w1a=w1a, b1a=b1a, w1b=w1b, b1b=b1b, w2a=w2a, b2a=b2a, w2b=w2b,
        b2b=b2b, w3a=w3a, b3a=b3a, w3b=w3b, b3b=b3b).items()}

    n_cores = NCORES
    cfg, common, per_core, meta = _prep(pos, edge_index, W, n_cores)
    key = tuple(sorted(cfg.items()))
    if key not in _CACHE:
        _CACHE[key] = _build_nc(**cfg)
    nc = _CACHE[key]

    in_maps = [dict(common, **per_core[c]) for c in range(n_cores)]
    res = bass_utils.run_bass_kernel_spmd(
        nc, in_maps, core_ids=list(range(n_cores)))

    n_loc = meta["n_loc"]
    h3 = np.concatenate(
        [np.asarray(res.results[c]["hT3"])[:, :n_loc].T
         for c in range(n_cores)], 0).astype(np.float32)
    if meta["deg0"] is not None and meta["deg0"].any():
        h3[meta["deg0"]] = 0.0

    kernel._last_h3 = h3
    nb = 64 if pos.shape[0] == N else int(batch.max()) + 1
    sums = np.zeros((nb, 128), np.float64)
    np.add.at(sums, batch, h3.astype(np.float64))
    counts = np.bincount(batch, minlength=nb).astype(np.float64)
    pooled = (sums / np.maximum(counts, 1.0)[:, None]).astype(np.float32)
    out = pooled @ np.asarray(wr1, np.float32) + np.asarray(br1, np.float32)
    out = out @ np.asarray(wr2, np.float32) + np.asarray(br2, np.float32)
    out = 1.0 / (1.0 + np.exp(-out))
    return out.squeeze(-1).astype(np.float32)


# revision 8
# speedup vs baseline: 1377.6398x; 1377.6398x over previous
"""Trainium2 Bass kernel for nn_BackbonePointNet (3-layer PointNet-style GNN).

Sharding: destination nodes across 8 cores (12.5K nodes / 200K edges each).
Per layer l (factored edge MLP):
    pre(e) = u_l[src_e] + v_l[dst_e]        (v holds the negated dst part)
    msg(e) = relu(pre) @ wb_l               (bias bb_l folded in after max)
    h(i)   = relu(max_{e->i} msg(e) + bb_l)
with u_l = concat(h_{l-1}, 1) @ [wa_h; ba] + pos @ wa_p  computed per-core
for local nodes then AllGather-replicated in bf16.  Edge phase per
1024-edge macro-tile: 8 indirect-DMA row gathers of u[src] (128 rows each),
transposed into PSUM via identity matmuls (accumulating on top of the
v-selector matmul), ACT relu -> bf16, second linear on PE, segment max via
strided tensor_reduce written straight into the transposed h accumulator,
which directly feeds the next layer's u matmuls.  Pooling (segment mean,
sorted batch) and the 2-layer regressor + sigmoid are O(B*C) and run on
host in f64/f32.
"""

import numpy as np
import ml_dtypes

N = 100_000
E = 16 * N
B = 64
NCORES = 8

_BF = ml_dtypes.bfloat16
_CACHE = {}


# --------------------------------------------------------------------------
# device program
# --------------------------------------------------------------------------

def _build_nc(n_nodes, n_loc_pad, d_grp, et, n_cores, collectives=True):
    from concourse import bass, mybir, tile  # noqa: F401
    import concourse.bacc as bacc

    BF16 = mybir.dt.bfloat16
    F32 = mybir.dt.float32
    AF = mybir.ActivationFunctionType

    e_loc = n_loc_pad * d_grp
    n_tiles = e_loc // et
    npt = et // d_grp                      # nodes per macro tile
    chunks = et // 128
    n_loc = n_nodes // n_cores

    nc = bacc.Bacc("TRN2", target_bir_lowering=False, debug=False,
                   num_devices=n_cores)

    # ---- external inputs ----
    u1_full = nc.dram_tensor("u1_full", [n_nodes, 64], BF16, kind="ExternalInput")
    gidx = nc.dram_tensor("gidx", [128, n_tiles * chunks], mybir.dt.int32,
                          kind="ExternalInput")
    # packed bf16 constants: [id128 | sel | w2h(65) | w2p(3) | w3h(65) |
    #                         w3p(3) | wb1 | wb2 | wb3]
    CW = 128 + et + 64 + 64 + 128 + 128 + 64 + 64 + 128
    cblob = nc.dram_tensor("cblob", [128, CW], BF16, kind="ExternalInput")
    fblob = nc.dram_tensor("fblob", [128, 3], F32, kind="ExternalInput")
    posT = nc.dram_tensor("posT", [3, n_loc_pad], BF16, kind="ExternalInput")
    nblk_v = (n_tiles + 1) // 2
    v1 = nc.dram_tensor("v1", [128, nblk_v * 64], BF16, kind="ExternalInput")
    v2 = nc.dram_tensor("v2", [128, nblk_v * 64], BF16, kind="ExternalInput")
    v3 = nc.dram_tensor("v3", [128, nblk_v * 128], BF16, kind="ExternalInput")
    hT3_out = nc.dram_tensor("hT3", [128, n_loc_pad], F32, kind="ExternalOutput")

    # internal dram for u slices / replicated tables
    u2_slice = nc.dram_tensor("u2_slice", [n_loc, 64], BF16, kind="Internal")
    u3_slice = nc.dram_tensor("u3_slice", [n_loc, 128], BF16, kind="Internal")
    u2_full = nc.dram_tensor("u2_full", [n_nodes, 64], BF16, kind="Internal",
                             addr_space="Shared")
    u3_full = nc.dram_tensor("u3_full", [n_nodes, 128], BF16, kind="Internal",
                             addr_space="Shared")

    with tile.TileContext(nc) as tc:
        with tc.tile_pool(name="const", bufs=1) as cp, \
             tc.tile_pool(name="gath", bufs=16) as gp, \
             tc.tile_pool(name="work", bufs=3) as wp, \
             tc.tile_pool(name="out", bufs=2) as op, \
             tc.tile_pool(name="hbuf", bufs=1) as hp, \
             tc.tile_pool(name="psum", bufs=2, space="PSUM") as pp:

            # ---- resident constants ----
            gidx_t = cp.tile([128, n_tiles * chunks], mybir.dt.int32)
            nc.sync.dma_start(out=gidx_t[:], in_=gidx[:])
            cb = cp.tile([128, CW], BF16)
            nc.sync.dma_start(out=cb[:], in_=cblob[:])
            fb = cp.tile([128, 3], F32)
            nc.sync.dma_start(out=fb[:], in_=fblob[:])
            posT_t = cp.tile([3, n_loc_pad], BF16)
            nc.sync.dma_start(out=posT_t[:], in_=posT[:])

            o_id = 0
            o_sel = o_id + 128
            o_w2h = o_sel + et
            o_w2p = o_w2h + 64
            o_w3h = o_w2p + 64
            o_w3p = o_w3h + 128
            o_wb1 = o_w3p + 128
            o_wb2 = o_wb1 + 64
            o_wb3 = o_wb2 + 64
            id_ap = cb[:, o_id:o_id + 128]
            w_ap = {
                "w2h": cb[0:65, o_w2h:o_w2h + 64],
                "w2p": cb[0:3, o_w2p:o_w2p + 64],
                "w3h": cb[0:65, o_w3h:o_w3h + 128],
                "w3p": cb[0:3, o_w3p:o_w3p + 128],
                "wb1": cb[0:64, o_wb1:o_wb1 + 64],
                "wb2": cb[0:64, o_wb2:o_wb2 + 64],
                "wb3": cb[0:128, o_wb3:o_wb3 + 128],
            }
            bb_ap = {"bb1": fb[0:64, 0:1], "bb2": fb[0:64, 1:2],
                     "bb3": fb[0:128, 2:3]}

            v_t = {}
            nblk = (n_tiles + 1) // 2
            for name, h, c in (("v1", v1, 64), ("v2", v2, 64), ("v3", v3, 128)):
                t = cp.tile([128, nblk * c], BF16, tag=name)
                nc.sync.dma_start(out=t[:], in_=h[:])
                v_t[name] = (t, c)

            hT1 = hp.tile([65, n_loc_pad], BF16, tag="hT1")
            hT2 = hp.tile([65, n_loc_pad], BF16, tag="hT2")
            hTr = hp.tile([128, n_loc_pad], BF16, tag="hTraw")
            nc.vector.memset(hT1[64:65, :], 1.0)
            nc.vector.memset(hT2[64:65, :], 1.0)

            def edge_phase(u_src_ap, v_name, wb_name, c_in, c_out):
                vt, vc = v_t[v_name]
                vv = vt[:].rearrange("p (m c) -> p m c", c=vc)
                for t in range(n_tiles):
                    gts = []
                    for c in range(chunks):
                        gt = gp.tile([128, c_in], BF16, tag="g")
                        nc.gpsimd.indirect_dma_start(
                            out=gt[:], out_offset=None,
                            in_=u_src_ap,
                            in_offset=bass.IndirectOffsetOnAxis(
                                ap=gidx_t[:, t * chunks + c: t * chunks + c + 1],
                                axis=0),
                        )
                        gts.append(gt)
                    pre = pp.tile([c_in, et], F32, tag="pre", space="PSUM")
                    r0 = (t % 2) * 64
                    m0 = t // 2
                    vslice = vv[r0:r0 + npt, m0:m0 + 1, :]
                    cpH = chunks // 2
                    for h in range(2):
                        sel_ap = cb[r0:r0 + npt, o_sel + h * 512:o_sel + h * 512 + 512]
                        nc.tensor.matmul(out=pre[:, h * 512:h * 512 + 512],
                                         lhsT=vslice, rhs=sel_ap,
                                         start=True, stop=False)
                        for j in range(cpH):
                            c = h * cpH + j
                            nc.tensor.matmul(
                                out=pre[:, c * 128:(c + 1) * 128],
                                lhsT=gts[c][:], rhs=id_ap,
                                start=False, stop=(j == cpH - 1),
                                skip_group_check=True)
                    prs = wp.tile([128, et], BF16, tag="prs")
                    nc.scalar.activation(out=prs[:c_in, :], in_=pre[:],
                                         func=AF.Relu)
                    msg = pp.tile([c_out, et], F32, tag="msg", space="PSUM")
                    for h in range(2):
                        nc.tensor.matmul(out=msg[:, h * 512:h * 512 + 512],
                                         lhsT=w_ap[wb_name],
                                         rhs=prs[:c_in, h * 512:h * 512 + 512],
                                         start=True, stop=True)
                    nc.vector.tensor_reduce(
                        out=hTr[:c_out, t * npt:(t + 1) * npt],
                        in_=msg[:].rearrange("p (n k) -> p n k", k=d_grp),
                        axis=mybir.AxisListType.X, op=mybir.AluOpType.max)

            def bias_relu(c_out, bb_name, dst):
                nc.scalar.activation(out=dst[:c_out, :], in_=hTr[:c_out, :],
                                     func=AF.Relu, bias=bb_ap[bb_name],
                                     scale=1.0)

            def u_phase(hT, wh_name, wp_name, c_out, u_slice):
                for m in range(n_loc_pad // 128):
                    ps = pp.tile([128, et], F32, tag="pre", space="PSUM")
                    nc.tensor.matmul(
                        out=ps[:, :c_out],
                        lhsT=hT[:, m * 128:(m + 1) * 128],
                        rhs=w_ap[wh_name], start=True, stop=False)
                    nc.tensor.matmul(
                        out=ps[:, :c_out],
                        lhsT=posT_t[:, m * 128:(m + 1) * 128],
                        rhs=w_ap[wp_name], start=False, stop=True,
                        skip_group_check=True)
                    us = wp.tile([128, 128], BF16, tag="us")
                    nc.scalar.activation(out=us[:, :c_out], in_=ps[:, :c_out],
                                         func=AF.Copy)
                    lo = m * 128
                    hi = min((m + 1) * 128, n_loc)
                    if hi > lo:
                        nc.sync.dma_start(out=u_slice[lo:hi, :],
                                          in_=us[:hi - lo, :c_out])

            # ---------------- layer 1 ----------------
            edge_phase(u1_full[:], "v1", "wb1", 64, 64)
            bias_relu(64, "bb1", hT1)

            # ---------------- layer 2 ----------------
            u_phase(hT1, "w2h", "w2p", 64, u2_slice)
            if collectives:
                nc.gpsimd.collective_compute(
                    "AllGather", mybir.AluOpType.bypass,
                    replica_groups=[list(range(n_cores))],
                    ins=[u2_slice[:]], outs=[u2_full[:]])
            else:
                nc.sync.dma_start(out=u2_full[0:n_loc, :], in_=u2_slice[:])
            edge_phase(u2_full[:], "v2", "wb2", 64, 64)
            bias_relu(64, "bb2", hT2)

            # ---------------- layer 3 ----------------
            u_phase(hT2, "w3h", "w3p", 128, u3_slice)
            if collectives:
                nc.gpsimd.collective_compute(
                    "AllGather", mybir.AluOpType.bypass,
                    replica_groups=[list(range(n_cores))],
                    ins=[u3_slice[:]], outs=[u3_full[:]])
            else:
                nc.sync.dma_start(out=u3_full[0:n_loc, :], in_=u3_slice[:])
            edge_phase(u3_full[:], "v3", "wb3", 128, 128)

            # final h3 = relu(raw + bb3) -> f32 output, in strips
            ns = 8
            strip = n_loc_pad // ns
            for s in range(ns):
                h3t = op.tile([128, strip], F32, tag="h3")
                nc.scalar.activation(out=h3t[:], in_=hTr[:, s * strip:(s + 1) * strip],
                                     func=AF.Relu, bias=bb_ap["bb3"], scale=1.0)
                nc.sync.dma_start(out=hT3_out[:, s * strip:(s + 1) * strip],
                                  in_=h3t[:])

    nc.compile()
    return nc


# --------------------------------------------------------------------------
# host side
# --------------------------------------------------------------------------

def _next_pow2_ge(x, lo=16):
    d = lo
    while d < x:
        d *= 2
    return d


def _prep(pos, edge_index, weights, n_cores):
    n_nodes = pos.shape[0]
    src = edge_index[0].astype(np.int64)
    dst = edge_index[1].astype(np.int64)
    e_tot = src.shape[0]

    canonical = (e_tot == 16 * n_nodes) and np.array_equal(
        dst, np.repeat(np.arange(n_nodes, dtype=np.int64), e_tot // n_nodes))

    if canonical and e_tot // n_nodes == 16:
        d_grp = 16
        slot_src = src.reshape(n_nodes, 16)
        deg0 = None
    else:
        order = np.argsort(dst, kind="stable")
        s_sorted = src[order]
        counts = np.bincount(dst, minlength=n_nodes)
        d_grp = _next_pow2_ge(int(counts.max()) if e_tot else 16)
        starts = np.concatenate([[0], np.cumsum(counts)])
        slot_src = np.zeros((n_nodes, d_grp), np.int64)
        idx = np.arange(d_grp)
        for i in range(n_nodes):
            c = counts[i]
            if c:
                row = s_sorted[starts[i]:starts[i] + c]
                slot_src[i] = row[idx % c]
        deg0 = counts == 0

    n_loc = n_nodes // n_cores
    et = 1024
    npt = et // d_grp
    n_loc_pad = int(np.ceil(n_loc / 128) * 128)

    sel_np = np.zeros((npt, et), np.float32)
    for k in range(npt):
        sel_np[k, k * d_grp:(k + 1) * d_grp] = 1.0

    w = weights
    u1_full = (pos @ (w['w1a'][:3] + w['w1a'][3:6]) + w['b1a']).astype(_BF)

    CW = 128 + et + 64 + 64 + 128 + 128 + 64 + 64 + 128
    cblob = np.zeros((128, CW), np.float32)
    o = 0
    cblob[:128, o:o + 128] = np.eye(128); o += 128
    cblob[:, o:o + et] = np.tile(sel_np, (128 // npt, 1)); o += et
    cblob[:65, o:o + 64] = np.concatenate([w['w2a'][:64], w['b2a'][None]], 0); o += 64
    cblob[:3, o:o + 64] = w['w2a'][64:67]; o += 64
    cblob[:65, o:o + 128] = np.concatenate([w['w3a'][:64], w['b3a'][None]], 0); o += 128
    cblob[:3, o:o + 128] = w['w3a'][64:67]; o += 128
    cblob[:64, o:o + 64] = w['w1b']; o += 64
    cblob[:64, o:o + 64] = w['w2b']; o += 64
    cblob[:128, o:o + 128] = w['w3b']; o += 128
    fblob = np.zeros((128, 3), np.float32)
    fblob[:64, 0] = w['b1b']
    fblob[:64, 1] = w['b2b']
    fblob[:128, 2] = w['b3b']

    common = dict(u1_full=u1_full, cblob=cblob.astype(_BF), fblob=fblob)

    chunks = et // 128
    n_tiles = n_loc_pad * d_grp // et
    per_core = []
    for c in range(n_cores):
        lo = c * n_loc
        pos_l = np.zeros((n_loc_pad, 3), np.float32)
        pos_l[:n_loc] = pos[lo:lo + n_loc]
        posT = pos_l.T.astype(_BF)
        npt_ = et // d_grp
        ntl = n_loc_pad * d_grp // et
        nblk = (ntl + 1) // 2

        def vpack(v):
            c_ = v.shape[1]
            out = np.zeros((128, nblk, c_), np.float32)
            for t in range(ntl):
                rows = v[t * npt_:(t + 1) * npt_]
                out[(t % 2) * 64:(t % 2) * 64 + npt_, t // 2, :] = rows
            return np.ascontiguousarray(out.reshape(128, nblk * c_)).astype(_BF)

        vs = {
            "v1": vpack(-(pos_l @ w['w1a'][3:6])),
            "v2": vpack(-(pos_l @ w['w2a'][64:67])),
            "v3": vpack(-(pos_l @ w['w3a'][64:67])),
        }
        ss = np.zeros((n_loc_pad, d_grp), np.int64)
        ss[:n_loc] = slot_src[lo:lo + n_loc]
        gidx = ss.reshape(-1).reshape(n_tiles, chunks, 128).transpose(2, 0, 1)
        gidx = np.ascontiguousarray(gidx.reshape(128, n_tiles * chunks),
                                    dtype=np.int32)
        per_core.append(dict(posT=posT, gidx=gidx, **vs))

    cfg = dict(n_nodes=n_nodes, n_loc_pad=n_loc_pad, d_grp=d_grp, et=et,
               n_cores=n_cores)
    meta = dict(n_loc=n_loc, deg0=deg0)
    return cfg, common, per_core, meta


def kernel(pos, edge_index, batch, timestep,
           w1a, b1a, w1b, b1b, w2a, b2a, w2b, b2b,
           w3a, b3a, w3b, b3b, wr1, br1, wr2, br2):
    from concourse import bass_utils

    pos = np.asarray(pos, np.float32)
    edge_index = np.asarray(edge_index, np.int32)
    batch = np.asarray(batch, np.int32)
    W = {k: np.asarray(v, np.float32) for k, v in dict(
        w1a=w1a, b1a=b1a, w1b=w1b, b1b=b1b, w2a=w2a, b2a=b2a, w2b=w2b,
        b2b=b2b, w3a=w3a, b3a=b3a, w3b=w3b, b3b=b3b).items()}

    n_cores = NCORES
    cfg, common, per_core, meta = _prep(pos, edge_index, W, n_cores)
    key = tuple(sorted(cfg.items()))
    if key not in _CACHE:
        _CACHE[key] = _build_nc(**cfg)
    nc = _CACHE[key]

    in_maps = [dict(common, **per_core[c]) for c in range(n_cores)]
    res = bass_utils.run_bass_kernel_spmd(
        nc, in_maps, core_ids=list(range(n_cores)))

    n_loc = meta["n_loc"]
    h3 = np.concatenate(
        [np.asarray(res.results[c]["hT3"])[:, :n_loc].T
         for c in range(n_cores)], 0).astype(np.float32)
    if meta["deg0"] is not None and meta["deg0"].any():
        h3[meta["deg0"]] = 0.0

    kernel._last_h3 = h3
    nb = 64 if pos.shape[0] == N else int(batch.max()) + 1
    sums = np.zeros((nb, 128), np.float64)
    np.add.at(sums, batch, h3.astype(np.float64))
    counts = np.bincount(batch, minlength=nb).astype(np.float64)
    pooled = (sums / np.maximum(counts, 1.0)[:, None]).astype(np.float32)
    out = pooled @ np.asarray(wr1, np.float32) + np.asarray(br1, np.float32)
    out = out @ np.asarray(wr2, np.float32) + np.asarray(br2, np.float32)
    out = 1.0 / (1.0 + np.exp(-out))
    return out.squeeze(-1).astype(np.float32)
